# revision 68
# baseline (speedup 1.0000x reference)
"""Trainium2 Bass kernel for the binary-conv BasicBlock (dense_cnn).

Computation (forward values only):
  A1   = sign(x + b11)
  out1 = x + bn1(conv3x3(A1, binw(w3)))          binw(w) = mean|w| * sign(w)
  o1   = prelu(out1 + b12, a1) + b13
  A2   = sign(o1 + b21)
  out2 = bn2(conv1x1(A2, binw(w1))) + o1
  out  = prelu(out2 + b22, a2) + b23

Strategy: data-parallel over batch, 4 images/core on 8 cores.  Per core:
  - A1 shipped from host as H1 = (x+b11 >= 0) in {0,1} fp8 (padded+halo
    layout); conv(2H-1) border correction M1 is folded into xprep on host.
  - xprep = x + (ch1+b12) - sh1*M1 shipped as bf16 (halves input DMA).
  - rows are stored at 57-pitch: one shared zero column serves as both
    the right-pad of row r and the left-pad of row r+1, so each 8-row
    conv1 tile is a contiguous 456-col matmul with only 8 garbage cols.
  - conv3x3 = 9 shifted fp8 DoubleRow matmuls per row-aligned tile;
    conv1x1 = one DR matmul per compact 448-col tile.
  - DVE stt: t1 = psum1*(2sh1) + xprep   (bf16, paired 2-bank PSUM drains)
  - downstream of t1 everything is compact 3136-px layout (the garbage
    pad column is sliced off by strided interior reads)
  - A2 binarization split across engines with per-half weight scaling:
      kc0: DVE is_ge -> {0,1}, conv2 weights for this half are +-2
      kc1: ACT Sign  -> +-1,  conv2 weights for this half are +-1
    (thr = prelu^-1(-(b13+b21)); requires a1 >= 0, else numpy fallback;
     the {0,1} half's rowsum correction is folded into K2')
  - ACT Prelu: p1 = prelu(t1, a1)
  - conv2 (1x1, fp8 DR): DVE stt t2 = psum2*sh2 + p1, then
    ACT Prelu (bias=K2', alpha=a2) -> out bf16; b23 + fp32 on host.
  Schedule: the PE is the bottleneck (fp8 peak ~157 TF/s => ~110us of
  matmul per core), so conv2 pairs are interleaved into conv1 slots one
  pair-slot behind their binarized data; per-pair binarize keeps the
  dependency chain ahead of the PE and the tail drains in small steps.
  Input DMAs issue from the idle Pool queue (Sync issue costs ~0.6us each).
"""

import numpy as np
import ml_dtypes

C = 256
H = W = 56
PH = 57                    # row pitch: one shared zero col between rows
NPIX = 58 * PH             # 3306: top-halo row + 56 data rows + bottom halo
IP = 56 * PH               # 3192: the 7x456 window outputs per mc
OP = H * W                 # 3136 compact output pixels
HOFF = 16                  # halo before H1 data (16B aligned block)
H1BLK = 3328               # 16 + 3306 + 6, multiple of 16
NT = 7                     # row tiles per image
TN = 8 * PH                # 456 columns per tile (8 rows)
BPC = 4                    # images per core
NCORES = 8
EPS = 1e-5

_CACHE = {}


def _split_drain_waits(m, max_waits=1):
    """This toolchain's walrus rejects instructions carrying more than ~1-2
    sync waits; hoist extra waits onto preceding single-wait EventSemaphore
    ops on the same engine (semantically identical: the engine blocks on
    each wait in sequence before executing the instruction)."""
    import copy as _copy
    from concourse import mybir

    new_module = _copy.replace(m, functions=[])
    for function in m.functions:
        new_function = _copy.replace(function, blocks=[])
        new_function.set_allocations_from_list(function.allocations)
        for block in function.blocks:
            out = []
            for inst in block.instructions:
                si = inst.sync_info
                if si is not None and len(si.on_wait) > max_waits:
                    waits = list(si.on_wait)
                    keep = waits[:max_waits] if not isinstance(
                        inst, mybir.InstDrain) else []
                    hoist = waits[len(keep):]
                    for i, wt in enumerate(hoist):
                        out.append(
                            mybir.InstEventSemaphore(
                                name=f"{inst.name}-wsplit{i}",
                                opcode="EventSemaphore",
                                engine=inst.engine,
                                sync_info=mybir.SyncInfo(on_wait=[wt], on_update=[]),
                            )
                        )
                    inst.sync_info = mybir.SyncInfo(
                        on_wait=keep, on_update=list(si.on_update)
                    )
                out.append(inst)
            new_block = _copy.replace(block, instructions=out)
            new_function.blocks.append(new_block)
        new_module.functions.append(new_function)
    return new_module


def build_nc():
    """Build (once) the per-core Bass program."""
    if "nc" in _CACHE:
        return _CACHE["nc"]
    import concourse.bass as bass
    import concourse.tile as tile
    from concourse import mybir

    Alu = mybir.AluOpType
    AF = mybir.ActivationFunctionType
    f32 = mybir.dt.float32
    bf16 = mybir.dt.bfloat16
    fp8 = mybir.dt.float8e4
    DR = mybir.MatmulPerfMode.DoubleRow

    nc = bass.Bass(trn_type="TRN2", debug=False)
    x_d = nc.dram_tensor("xprep", [BPC, 2, 128, NPIX], bf16, kind="ExternalInput")
    h_d = nc.dram_tensor("h1", [BPC, 128, 2 * H1BLK], fp8, kind="ExternalInput")
    w3_d = nc.dram_tensor("w3f", [128, 9 * 2 * 2 * 128], fp8, kind="ExternalInput")
    w1_d = nc.dram_tensor("w1f", [128, 2 * 2 * 128], fp8, kind="ExternalInput")
    c_d = nc.dram_tensor("consts", [2, 128, 8], f32, kind="ExternalInput")
    o_d = nc.dram_tensor("out", [BPC, 2, 128, OP], bf16, kind="ExternalOutput")

    with tile.TileContext(nc) as tc:
        with (
            tc.tile_pool(name="wpool", bufs=1) as wpool,
            tc.tile_pool(name="xpool", bufs=2) as xpool,
            tc.tile_pool(name="hpool", bufs=2) as hpool,
            tc.tile_pool(name="t1pool", bufs=2) as t1pool,
            tc.tile_pool(name="p1pool", bufs=2) as p1pool,
            tc.tile_pool(name="a2pool", bufs=2) as a2pool,
            tc.tile_pool(name="t2pool", bufs=2) as t2pool,
            tc.tile_pool(name="opool", bufs=2) as opool,
            tc.tile_pool(name="ps1", bufs=2, space="PSUM") as ps1p,
            tc.tile_pool(name="ps2", bufs=2, space="PSUM") as ps2p,
        ):
            # ---- constants / weights (resident) ----
            w3sb = wpool.tile([128, 9 * 2 * 2 * 128], fp8, tag="w3")
            wcut = 5 * 2 * 2 * 128
            nc.sync.dma_start(w3sb[:, :wcut], w3_d.ap()[:, :wcut])
            nc.sync.dma_start(w3sb[:, wcut:], w3_d.ap()[:, wcut:])
            w1sb = wpool.tile([128, 2 * 2 * 128], fp8, tag="w1")
            nc.sync.dma_start(w1sb[:], w1_d.ap())
            # [p, (sh mc), 2, m] / [p, mc, 2, m] views for DoubleRow lhsT
            w3v = w3sb[:].rearrange("p (g two m) -> p g two m", two=2, m=128)
            w1v = w1sb[:].rearrange("p (g two m) -> p g two m", two=2, m=128)
            csb = []
            for kc in range(2):
                ct = wpool.tile([128, 8], f32, tag=f"c_{kc}")
                nc.sync.dma_start(ct[:], c_d.ap()[kc])
                csb.append(ct)

            def cc(kc, j):
                # slots: 0=2sh1 1=thr 2=K2' 3=a1 4=a2 5=sh2 6=-thr
                return csb[kc][:, j : j + 1]

            # PE pstate pre-ramp: the clock needs ~3us of continuous busy
            # to reach 2.4GHz; burn the input-DMA wait on dummy matmuls
            # over a zeroed scratch tile (results never read)
            scr = wpool.tile([128, 512], bf16, tag="scr")
            nc.vector.memset(scr[:], 0.0)
            psd = ps1p.tile([128, 1024], f32, tag="ps1")
            for _ in range(5):
                nc.tensor.matmul(
                    psd[:, :TN], scr[:, 0:128], scr[:, :TN],
                    start=True, stop=True,
                )

            xts = [None] * BPC
            hts = [None] * BPC
            t1ts = [None] * BPC
            p1ts = [None] * BPC
            a2ts = [None] * BPC
            t2ts = [None] * BPC

            def load(img):
                # inputs issued from the (otherwise idle) Pool queue so the
                # Sync queue's ~0.6us-per-issue cost never gates compute;
                # both kc halves ride one 2-dim-AP DMA per chunk.
                # h1 first: the matmuls need it before xprep is touched.
                ht = hpool.tile([128, 2 * H1BLK], fp8, tag="h", name=f"h{img}")
                hv = ht[:].rearrange("p (two w) -> p two w", two=2)
                hdv = h_d.ap()[img].rearrange("p (two w) -> p two w", two=2)
                hb = (
                    [0, HOFF + 1056, HOFF + 1984, H1BLK]
                    if img == 0 else [0, H1BLK]
                )
                for b0, b1 in zip(hb, hb[1:]):
                    nc.gpsimd.dma_start(hv[:, :, b0:b1], hdv[:, :, b0:b1])
                xt = xpool.tile([128, 2 * NPIX], bf16, tag="x", name=f"x{img}")
                xv = xt[:].rearrange("p (two w) -> p two w", two=2)
                xdv = x_d.ap()[img]  # [2, 128, NPIX]
                xb = (
                    [0, PH + 2 * TN, PH + 4 * TN, PH + 6 * TN, NPIX]
                    if img == 0 else [0, NPIX]
                )
                for b0, b1 in zip(xb, xb[1:]):
                    nc.gpsimd.dma_start(
                        xv[:, :, b0:b1],
                        xdv.transpose([1, 0, 2])[:, :, b0:b1],
                    )
                xts[img], hts[img] = xt, ht

            def ge_tiles(img, t0, t1):
                c0, n = TN * t0, TN * (t1 - t0)
                rows = 8 * (t1 - t0)
                cc0, nc_ = 448 * t0, 448 * (t1 - t0)
                nc.vector.tensor_scalar(
                    a2ts[img][:, cc0 : cc0 + nc_].rearrange(
                        "p (h w) -> p h w", h=rows
                    ),
                    t1ts[img][:, c0 : c0 + n].rearrange(
                        "p (h w) -> p h w", h=rows
                    )[:, :, 1:57],
                    cc(0, 1), None, Alu.is_ge,
                )

            def binarize_tiles(img, t0, t1, skip_ge=False):
                # Sign first (it gates the interleaved conv2), then ge
                # (DVE) and prelu1; compact outputs
                c0, n = TN * t0, TN * (t1 - t0)
                rows = 8 * (t1 - t0)
                cc0, nc_ = 448 * t0, 448 * (t1 - t0)
                t1t = t1ts[img]

                def t1i(m):
                    return t1t[:, m * IP + c0 : m * IP + c0 + n].rearrange(
                        "p (h w) -> p h w", h=rows
                    )[:, :, 1:57]

                nc.scalar.activation(
                    a2ts[img][:, OP + cc0 : OP + cc0 + nc_].rearrange(
                        "p (h w) -> p h w", h=rows
                    ),
                    t1i(1), AF.Sign, bias=cc(1, 6),
                )
                if not skip_ge:
                    nc.vector.tensor_scalar(
                        a2ts[img][:, cc0 : cc0 + nc_].rearrange(
                            "p (h w) -> p h w", h=rows
                        ),
                        t1i(0), cc(0, 1), None, Alu.is_ge,
                    )
                for mc in range(2):
                    nc.scalar.activation(
                        p1ts[img][:, mc * OP + cc0 : mc * OP + cc0 + nc_]
                        .rearrange("p (h w) -> p h w", h=rows),
                        t1i(mc), AF.Prelu, alpha=cc(mc, 3),
                    )

            def conv1_pair(img, tp):
                # tiles 2tp(,2tp+1) share one 2-bank PSUM tile per mc so the
                # DVE drains 928 columns per instruction
                if tp == 0:
                    t1ts[img] = t1pool.tile(
                        [128, 2 * IP], bf16, tag="t1", name=f"t1_{img}"
                    )
                    a2ts[img] = a2pool.tile(
                        [128, 2 * OP], fp8, tag="a2", name=f"a2_{img}"
                    )
                    p1ts[img] = p1pool.tile(
                        [128, 2 * OP], bf16, tag="p1", name=f"p1_{img}"
                    )
                t1t = t1ts[img]
                hv = hts[img][:].rearrange("p (two w) -> p two w", two=2)
                tiles = [2 * tp] if 2 * tp + 1 >= NT else [2 * tp, 2 * tp + 1]
                c0 = TN * 2 * tp
                n = TN * len(tiles)
                # last image: mc1 first, its Sign gates the tail conv2
                for mc in ((1, 0) if img == BPC - 1 else (0, 1)):
                    ps = ps1p.tile([128, 1024], f32, tag="ps1")
                    for j, t in enumerate(tiles):
                        for sh in range(9):
                            kh, kw = divmod(sh, 3)
                            off = HOFF + (8 * t + kh) * PH + (kw - 1)
                            nc.tensor.matmul(
                                ps[:, 512 * j : 512 * j + TN],
                                w3v[:, sh * 2 + mc],
                                hv[:, :, off : off + TN],
                                start=(sh == 0),
                                stop=(sh == 8),
                                perf_mode=DR,
                            )
                    # t1 = psum*(2*sh1) + xprep (x + bn1 + b12 - sh1*M1 folded)
                    if len(tiles) == 2:
                        psv = ps[:].rearrange(
                            "p (two w) -> p two w", two=2
                        )[:, :, :TN]
                        t1v = t1t[:, mc * IP + c0 : mc * IP + c0 + n].rearrange(
                            "p (two w) -> p two w", two=2
                        )
                        xv = xts[img][
                            :, mc * NPIX + PH + c0 : mc * NPIX + PH + c0 + n
                        ].rearrange("p (two w) -> p two w", two=2)
                        nc.vector.scalar_tensor_tensor(
                            t1v, psv, cc(mc, 0), xv, Alu.mult, Alu.add
                        )
                    else:
                        nc.vector.scalar_tensor_tensor(
                            t1t[:, mc * IP + c0 : mc * IP + c0 + n],
                            ps[:, :TN], cc(mc, 0),
                            xts[img][:, mc * NPIX + PH + c0 :
                                      mc * NPIX + PH + c0 + n],
                            Alu.mult, Alu.add,
                        )

            ots = [None] * BPC

            TC = 448  # compact tile width (8 rows x 56)

            def conv2_pair(img, tp):
                if tp == 0:
                    t2ts[img] = t2pool.tile(
                        [128, 2 * OP], bf16, tag="t2", name=f"t2_{img}"
                    )
                    ots[img] = opool.tile(
                        [128, 2 * OP], bf16, tag="o", name=f"o_{img}"
                    )
                t2t = t2ts[img]
                a2v = a2ts[img][:].rearrange("p (two w) -> p two w", two=2)
                tiles = [2 * tp] if 2 * tp + 1 >= NT else [2 * tp, 2 * tp + 1]
                c0 = TC * 2 * tp
                n = TC * len(tiles)
                for mc in range(2):
                    ps = ps2p.tile([128, 1024], f32, tag="ps2")
                    for j, t in enumerate(tiles):
                        nc.tensor.matmul(
                            ps[:, 512 * j : 512 * j + TC],
                            w1v[:, mc],
                            a2v[:, :, TC * t : TC * t + TC],
                            start=True,
                            stop=True,
                            perf_mode=DR,
                        )
                    # t2 = psum*sh2 + p1 on DVE; prelu2 chunk follows (ACT)
                    if len(tiles) == 2:
                        psv = ps[:].rearrange(
                            "p (two w) -> p two w", two=2
                        )[:, :, :TC]
                        t2v = t2t[:, mc * OP + c0 : mc * OP + c0 + n].rearrange(
                            "p (two w) -> p two w", two=2
                        )
                        pv = p1ts[img][
                            :, mc * OP + c0 : mc * OP + c0 + n
                        ].rearrange("p (two w) -> p two w", two=2)
                        nc.vector.scalar_tensor_tensor(
                            t2v, psv, cc(mc, 5), pv, Alu.mult, Alu.add
                        )
                    else:
                        nc.vector.scalar_tensor_tensor(
                            t2t[:, mc * OP + c0 : mc * OP + c0 + n],
                            ps[:, :TC], cc(mc, 5),
                            p1ts[img][:, mc * OP + c0 : mc * OP + c0 + n],
                            Alu.mult, Alu.add,
                        )

            def prelu2_tiles(img, t0, t1):
                cc0, nc_ = 448 * t0, 448 * (t1 - t0)
                for mc in range(2):
                    # out = prelu(t2 + K2', a2); b23 on host
                    nc.scalar.activation(
                        ots[img][:, mc * OP + cc0 : mc * OP + cc0 + nc_],
                        t2ts[img][:, mc * OP + cc0 : mc * OP + cc0 + nc_],
                        AF.Prelu, bias=cc(mc, 2), alpha=cc(mc, 4),
                    )

            def dma_out_tiles(img, t0, t1):
                b0, b1 = 448 * t0, 448 * t1
                for mc in range(2):
                    nc.sync.dma_start(
                        o_d.ap()[img, mc][:, b0:b1],
                        ots[img][:, mc * OP + b0 : mc * OP + b1],
                    )

            load(0)
            NP_ = (NT + 1) // 2  # 4 pair-slots
            LAST = BPC - 1
            for img in range(BPC):
                if img + 1 < BPC:
                    load(img + 1)
                last = img == LAST
                for tp in range(NP_):
                    conv1_pair(img, tp)
                    binarize_tiles(img, 2 * tp, min(2 * tp + 2, NT),
                                   skip_ge=last and tp < 3)
                    # one conv2 pair per slot: slot 0 hosts the previous
                    # image's last pair, slots 1-3 host this image's pairs
                    # 0-2 one slot behind their binarized data
                    if tp == 0:
                        if img >= 1:
                            conv2_pair(img - 1, 3)
                            prelu2_tiles(img - 1, 6, NT)
                            dma_out_tiles(img - 1, 6, NT)
                    else:
                        conv2_pair(img, tp - 1)
                        prelu2_tiles(img, 2 * (tp - 1), 2 * tp)
                        if last:
                            # stream the last image's output out as it is
                            # produced so the exit barrier never waits on
                            # a bulk DMA drain
                            dma_out_tiles(img, 2 * (tp - 1), 2 * tp)
                        elif tp == 3:
                            dma_out_tiles(img, 0, 6)
                    if last and tp < 3:
                        # last image: ge after the slot's stt2 drains; its
                        # consumer is a slot away, the PSUM banks are not.
                        # (pair 3 keeps ge inline: the tail conv2 gates on
                        # it immediately)
                        ge_tiles(img, 2 * tp, min(2 * tp + 2, NT))
            conv2_pair(LAST, 3)
            prelu2_tiles(LAST, 6, NT)
            dma_out_tiles(LAST, 6, NT)

    _CACHE["nc"] = nc
    return nc


def _host_fold(w3, w1, b11, b12, b13, b21, b22, b23,
               g1, be1, m1, v1, g2, be2, m2, v2, a1, a2):
    f = np.float32
    s3 = np.mean(np.abs(w3), axis=(1, 2, 3)).astype(f)
    s1 = np.mean(np.abs(w1), axis=(1, 2, 3)).astype(f)
    inv1 = (g1 / np.sqrt(v1 + EPS)).astype(f)
    inv2 = (g2 / np.sqrt(v2 + EPS)).astype(f)
    sh1 = s3 * inv1
    ch1 = be1 - m1 * inv1
    sh2 = s1 * inv2
    ch2 = be2 - m2 * inv2
    K1 = (ch1 + b12).astype(f)

    sgn3 = np.sign(w3).astype(f)                     # [O, I, 3, 3]
    sgn1 = np.sign(w1).astype(f)
    # M1[c, i, j] = sum over in-bounds taps of rowsum3[c, kh, kw]
    rowsum3 = sgn3.sum(axis=1)                       # [C, 3, 3]
    M1 = np.zeros((C, H, W), f)
    for kh in range(3):
        for kw in range(3):
            ind = np.zeros((H, W), f)
            r0, r1 = max(0, 1 - kh), min(H - 1, H - kh) + 1
            c0, c1 = max(0, 1 - kw), min(W - 1, W - kw) + 1
            ind[r0:r1, c0:c1] = 1.0
            M1 += rowsum3[:, kh, kw][:, None, None] * ind[None]
    xadj = K1[:, None, None] - sh1[:, None, None] * M1   # [C, H, W]

    # A2 threshold: p1 >= -(b13+b21)  <=>  t1 >= thr (prelu inverse, a1>=0)
    u = (-(b13 + b21)).astype(f)
    safe_a1 = np.where(a1 > 0, a1, 1.0).astype(f)
    thr = np.where(u > 0, u, np.where(a1 > 0, u / safe_a1, f(-3e38)))
    # {0,1}-encoded kc0 half: conv(2H-1) correction = rowsum over kc0 inputs
    r1h = sgn1[:, :128].sum(axis=(1, 2, 3)).astype(f)
    K2p = (ch2 + b13 + b22 - sh2 * r1h).astype(f)

    fp8 = ml_dtypes.float8_e4m3
    # DoubleRow lhsT layout: [k, ((sh*2+mc)*2+i)*128+m] with i the K-half
    W3 = sgn3.astype(fp8)                                       # [O, I, 3, 3]
    W3 = W3.reshape(2, 128, 2, 128, 3, 3)                       # [mc,m,i,k,kh,kw]
    W3 = W3.transpose(3, 4, 5, 0, 2, 1)                         # [k,kh,kw,mc,i,m]
    W3f = np.ascontiguousarray(W3.reshape(128, 9 * 2 * 2 * 128))
    W1 = sgn1.reshape(2, 128, 2, 128).copy()                    # [mc, m, i, k]
    W1[:, :, 0, :] *= 2.0              # kc0 half is {0,1}-encoded: +-2
    W1 = W1.astype(fp8).transpose(3, 0, 2, 1)                   # [k, mc, i, m]
    W1f = np.ascontiguousarray(W1.reshape(128, 2 * 2 * 128))

    consts = np.zeros((2, 128, 8), f)
    for kc in range(2):
        sl = slice(kc * 128, (kc + 1) * 128)
        consts[kc, :, 0] = 2.0 * sh1[sl]
        consts[kc, :, 1] = thr[sl]
        consts[kc, :, 2] = K2p[sl]
        consts[kc, :, 3] = a1[sl]
        consts[kc, :, 4] = a2[sl]
        consts[kc, :, 5] = sh2[sl]
        consts[kc, :, 6] = -thr[sl]
    return W3f, W1f, consts, xadj


def _run(in_maps, trace=False, tmpdir=None, trace_kwargs={}):
    from concourse import bass_utils

    nc = build_nc()
    if not _CACHE.get("split"):
        # walrus workaround applied only for the HW path (CoreSim rejects
        # post-scheduling instruction edits)
        nc.m = _split_drain_waits(nc.m)
        _CACHE["split"] = True
    return bass_utils.run_bass_kernel_spmd(
        nc,
        in_maps,
        core_ids=list(range(NCORES)),
        trace=trace,
        tmpdir=tmpdir,
        trace_kwargs=trace_kwargs,
    )


def make_in_maps(x, w3, w1, **params):
    x = np.asarray(x, np.float32)
    params = {k: np.asarray(v, np.float32) for k, v in params.items()}
    W3f, W1f, consts, xadj = _host_fold(np.asarray(w3, np.float32),
                                        np.asarray(w1, np.float32), **params)
    _CACHE["b23"] = params["b23"]
    bf16 = ml_dtypes.bfloat16
    fp8 = ml_dtypes.float8_e4m3
    N = x.shape[0]

    xp = np.zeros((N, C, 58, PH), bf16)
    xp[:, :, 1:57, 1:57] = (x + xadj[None]).astype(bf16)
    x_prep = xp.reshape(N, 2, 128, NPIX)

    hp = np.zeros((N, C, 58, PH), fp8)
    hp[:, :, 1:57, 1:57] = (
        x + params["b11"][None, :, None, None] >= 0
    ).astype(fp8)
    harr = np.zeros((N, 128, 2, H1BLK), fp8)
    harr[:, :, :, HOFF : HOFF + NPIX] = hp.reshape(
        N, 2, 128, NPIX
    ).transpose(0, 2, 1, 3)
    harr = harr.reshape(N, 128, 2 * H1BLK)

    return [
        {
            "xprep": np.ascontiguousarray(x_prep[c * BPC : (c + 1) * BPC]),
            "h1": np.ascontiguousarray(harr[c * BPC : (c + 1) * BPC]),
            "w3f": W3f, "w1f": W1f, "consts": consts,
        }
        for c in range(NCORES)
    ]


def finish_out(arr):
    """Per-core raw out [BPC,2,128,OP] bf16 -> [BPC,C,H,W] fp32 (+b23)."""
    out = np.asarray(arr).reshape(BPC, C, H, W).astype(np.float32)
    return out + _CACHE["b23"][None, :, None, None]


def assemble_out(results):
    outs = [finish_out(results[c]["out"]) for c in range(NCORES)]
    return np.ascontiguousarray(np.concatenate(outs, axis=0))


def _fallback_numpy(x, w3, w1, b11, b12, b13, b21, b22, b23,
                    g1, be1, m1, v1, g2, be2, m2, v2, a1, a2):
    # Straightforward reference math in numpy; only used if an assumption of
    # the device kernel (prelu slope a1 >= 0) is violated.
    def cb(p):
        return p[None, :, None, None]

    def conv_np(a, w, pad):
        N, Ci, Hh, Ww = a.shape
        O, I, kh, kw = w.shape
        ap = np.pad(a, ((0, 0), (0, 0), (pad, pad), (pad, pad)))
        out = np.zeros((N, O, Hh, Ww), np.float32)
        wm = w.reshape(O, -1)
        for n in range(N):
            cols = np.empty((I * kh * kw, Hh * Ww), np.float32)
            idx = 0
            for i in range(I):
                for dh in range(kh):
                    for dw in range(kw):
                        cols[idx] = ap[n, i, dh : dh + Hh, dw : dw + Ww].ravel()
                        idx += 1
            out[n] = (wm @ cols).reshape(O, Hh, Ww)
        return out

    def bn(t, g, b, mm, v):
        inv = g / np.sqrt(v + EPS)
        return t * cb(inv) + cb(b - mm * inv)

    def prelu(t, a):
        return np.where(t > 0, t, cb(a) * t)

    s3 = np.mean(np.abs(w3), axis=(1, 2, 3), keepdims=True)
    s1 = np.mean(np.abs(w1), axis=(1, 2, 3), keepdims=True)
    o1 = conv_np(np.sign(x + cb(b11)), np.sign(w3) * s3, 1)
    o1 = x + bn(o1, g1, be1, m1, v1)
    o1 = prelu(o1 + cb(b12), a1) + cb(b13)
    o2 = conv_np(np.sign(o1 + cb(b21)), np.sign(w1) * s1, 0)
    o2 = bn(o2, g2, be2, m2, v2) + o1
    o2 = prelu(o2 + cb(b22), a2) + cb(b23)
    return o2.astype(np.float32)


def kernel(**inputs):
    inputs = {k: np.asarray(v) for k, v in inputs.items()}
    if (np.asarray(inputs["a1"], np.float32) < 0).any():
        return _fallback_numpy(**{k: np.asarray(v, np.float32)
                                  for k, v in inputs.items()})
    in_maps = make_in_maps(**inputs)
    res = _run(in_maps, trace=False)
    return assemble_out(res.results)


# revision 69
# speedup vs baseline: 1.1932x; 1.1932x over previous
"""Trainium2 Bass kernel for the binary-conv BasicBlock (dense_cnn).

Computation (forward values only):
  A1   = sign(x + b11)
  out1 = x + bn1(conv3x3(A1, binw(w3)))          binw(w) = mean|w| * sign(w)
  o1   = prelu(out1 + b12, a1) + b13
  A2   = sign(o1 + b21)
  out2 = bn2(conv1x1(A2, binw(w1))) + o1
  out  = prelu(out2 + b22, a2) + b23

Strategy: data-parallel over batch, 4 images/core on 8 cores.  Per core:
  - A1 shipped from host as H1 = (x+b11 >= 0) in {0,1} fp8 (padded+halo
    layout); conv(2H-1) border correction M1 is folded into xprep on host.
  - xprep = x + (ch1+b12) - sh1*M1 shipped as bf16 (halves input DMA).
  - rows are stored at 57-pitch: one shared zero column serves as both
    the right-pad of row r and the left-pad of row r+1, so each 8-row
    conv1 tile is a contiguous 456-col matmul with only 8 garbage cols.
  - conv3x3 = 9 shifted fp8 DoubleRow matmuls per row-aligned tile;
    conv1x1 = one DR matmul per compact 448-col tile.
  - DVE stt: t1 = psum1*(2sh1) + xprep   (bf16, paired 2-bank PSUM drains)
  - downstream of t1 everything is compact 3136-px layout (the garbage
    pad column is sliced off by strided interior reads)
  - A2 binarization split across engines with per-half weight scaling:
      kc0: DVE is_ge -> {0,1}, conv2 weights for this half are +-2
      kc1: ACT Sign  -> +-1,  conv2 weights for this half are +-1
    (thr = prelu^-1(-(b13+b21)); requires a1 >= 0, else numpy fallback;
     the {0,1} half's rowsum correction is folded into K2')
  - ACT Prelu: p1 = prelu(t1, a1)
  - conv2 (1x1, fp8 DR): DVE stt t2 = psum2*sh2 + p1, then
    ACT Prelu (bias=K2', alpha=a2) -> out bf16; b23 + fp32 on host.
  Schedule: the PE is the bottleneck (fp8 peak ~157 TF/s => ~110us of
  matmul per core), so conv2 pairs are interleaved into conv1 slots one
  pair-slot behind their binarized data; per-pair binarize keeps the
  dependency chain ahead of the PE and the tail drains in small steps.
  Input DMAs issue from the idle Pool queue (Sync issue costs ~0.6us each).
"""

import numpy as np
import ml_dtypes

C = 256
H = W = 56
PH = 57                    # row pitch: one shared zero col between rows
NPIX = 58 * PH             # 3306: top-halo row + 56 data rows + bottom halo
IP = 56 * PH               # 3192: the 7x456 window outputs per mc
OP = H * W                 # 3136 compact output pixels
HOFF = 16                  # halo before H1 data (16B aligned block)
H1BLK = 3328               # 16 + 3306 + 6, multiple of 16
NT = 7                     # row tiles per image
TN = 8 * PH                # 456 columns per tile (8 rows)
BPC = 4                    # images per core
NCORES = 8
EPS = 1e-5

_CACHE = {}


def _split_drain_waits(m, max_waits=1):
    """This toolchain's walrus rejects instructions carrying more than ~1-2
    sync waits; hoist extra waits onto preceding single-wait EventSemaphore
    ops on the same engine (semantically identical: the engine blocks on
    each wait in sequence before executing the instruction)."""
    import copy as _copy
    from concourse import mybir

    new_module = _copy.replace(m, functions=[])
    for function in m.functions:
        new_function = _copy.replace(function, blocks=[])
        new_function.set_allocations_from_list(function.allocations)
        for block in function.blocks:
            out = []
            for inst in block.instructions:
                si = inst.sync_info
                if si is not None and len(si.on_wait) > max_waits:
                    waits = list(si.on_wait)
                    keep = waits[:max_waits] if not isinstance(
                        inst, mybir.InstDrain) else []
                    hoist = waits[len(keep):]
                    for i, wt in enumerate(hoist):
                        out.append(
                            mybir.InstEventSemaphore(
                                name=f"{inst.name}-wsplit{i}",
                                opcode="EventSemaphore",
                                engine=inst.engine,
                                sync_info=mybir.SyncInfo(on_wait=[wt], on_update=[]),
                            )
                        )
                    inst.sync_info = mybir.SyncInfo(
                        on_wait=keep, on_update=list(si.on_update)
                    )
                out.append(inst)
            new_block = _copy.replace(block, instructions=out)
            new_function.blocks.append(new_block)
        new_module.functions.append(new_function)
    return new_module


def build_nc():
    """Build (once) the per-core Bass program."""
    if "nc" in _CACHE:
        return _CACHE["nc"]
    import concourse.bass as bass
    import concourse.tile as tile
    from concourse import mybir

    Alu = mybir.AluOpType
    AF = mybir.ActivationFunctionType
    f32 = mybir.dt.float32
    bf16 = mybir.dt.bfloat16
    fp8 = mybir.dt.float8e4
    DR = mybir.MatmulPerfMode.DoubleRow

    nc = bass.Bass(trn_type="TRN2", debug=False)
    x_d = nc.dram_tensor("xprep", [BPC, 2, 128, NPIX], bf16, kind="ExternalInput")
    h_d = nc.dram_tensor("h1", [BPC, 128, 2 * H1BLK], fp8, kind="ExternalInput")
    w3_d = nc.dram_tensor("w3f", [128, 9 * 2 * 2 * 128], fp8, kind="ExternalInput")
    w1_d = nc.dram_tensor("w1f", [128, 2 * 2 * 128], fp8, kind="ExternalInput")
    c_d = nc.dram_tensor("consts", [2, 128, 8], f32, kind="ExternalInput")
    o_d = nc.dram_tensor("out", [BPC, 2, 128, OP], bf16, kind="ExternalOutput")

    with tile.TileContext(nc) as tc:
        with (
            tc.tile_pool(name="wpool", bufs=1) as wpool,
            tc.tile_pool(name="xpool", bufs=2) as xpool,
            tc.tile_pool(name="hpool", bufs=2) as hpool,
            tc.tile_pool(name="t1pool", bufs=2) as t1pool,
            tc.tile_pool(name="p1pool", bufs=2) as p1pool,
            tc.tile_pool(name="a2pool", bufs=2) as a2pool,
            tc.tile_pool(name="t2pool", bufs=2) as t2pool,
            tc.tile_pool(name="opool", bufs=2) as opool,
            tc.tile_pool(name="ps1", bufs=2, space="PSUM") as ps1p,
            tc.tile_pool(name="ps2", bufs=2, space="PSUM") as ps2p,
        ):
            # ---- constants / weights (resident) ----
            w3sb = wpool.tile([128, 9 * 2 * 2 * 128], fp8, tag="w3")
            wcut = 5 * 2 * 2 * 128
            nc.sync.dma_start(w3sb[:, :wcut], w3_d.ap()[:, :wcut])
            nc.sync.dma_start(w3sb[:, wcut:], w3_d.ap()[:, wcut:])
            w1sb = wpool.tile([128, 2 * 2 * 128], fp8, tag="w1")
            nc.sync.dma_start(w1sb[:], w1_d.ap())
            # [p, (sh mc), 2, m] / [p, mc, 2, m] views for DoubleRow lhsT
            w3v = w3sb[:].rearrange("p (g two m) -> p g two m", two=2, m=128)
            w1v = w1sb[:].rearrange("p (g two m) -> p g two m", two=2, m=128)
            csb = []
            for kc in range(2):
                ct = wpool.tile([128, 8], f32, tag=f"c_{kc}")
                nc.sync.dma_start(ct[:], c_d.ap()[kc])
                csb.append(ct)

            def cc(kc, j):
                # slots: 0=2sh1 1=thr 2=K2' 3=a1 4=a2 5=sh2 6=-thr
                return csb[kc][:, j : j + 1]

            xts = [None] * BPC
            hts = [None] * BPC
            t1ts = [None] * BPC
            p1ts = [None] * BPC
            a2ts = [None] * BPC
            t2ts = [None] * BPC

            def load(img):
                # inputs issued from the (otherwise idle) Pool queue so the
                # Sync queue's ~0.6us-per-issue cost never gates compute;
                # both kc halves ride one 2-dim-AP DMA per chunk.
                # h1 first: the matmuls need it before xprep is touched.
                ht = hpool.tile([128, 2 * H1BLK], fp8, tag="h", name=f"h{img}")
                hv = ht[:].rearrange("p (two w) -> p two w", two=2)
                hdv = h_d.ap()[img].rearrange("p (two w) -> p two w", two=2)
                hb = (
                    [0, HOFF + 1056, HOFF + 1984, H1BLK]
                    if img == 0 else [0, H1BLK]
                )
                for b0, b1 in zip(hb, hb[1:]):
                    nc.gpsimd.dma_start(hv[:, :, b0:b1], hdv[:, :, b0:b1])
                xt = xpool.tile([128, 2 * NPIX], bf16, tag="x", name=f"x{img}")
                xv = xt[:].rearrange("p (two w) -> p two w", two=2)
                xdv = x_d.ap()[img]  # [2, 128, NPIX]
                xb = (
                    [0, PH + 2 * TN, PH + 4 * TN, PH + 6 * TN, NPIX]
                    if img == 0 else [0, NPIX]
                )
                for b0, b1 in zip(xb, xb[1:]):
                    nc.gpsimd.dma_start(
                        xv[:, :, b0:b1],
                        xdv.transpose([1, 0, 2])[:, :, b0:b1],
                    )
                xts[img], hts[img] = xt, ht

            def ge_tiles(img, t0, t1):
                c0, n = TN * t0, TN * (t1 - t0)
                rows = 8 * (t1 - t0)
                cc0, nc_ = 448 * t0, 448 * (t1 - t0)
                nc.vector.tensor_scalar(
                    a2ts[img][:, cc0 : cc0 + nc_].rearrange(
                        "p (h w) -> p h w", h=rows
                    ),
                    t1ts[img][:, c0 : c0 + n].rearrange(
                        "p (h w) -> p h w", h=rows
                    )[:, :, 1:57],
                    cc(0, 1), None, Alu.is_ge,
                )

            def binarize_tiles(img, t0, t1, skip_ge=False):
                # Sign first (it gates the interleaved conv2), then ge
                # (DVE) and prelu1; compact outputs
                c0, n = TN * t0, TN * (t1 - t0)
                rows = 8 * (t1 - t0)
                cc0, nc_ = 448 * t0, 448 * (t1 - t0)
                t1t = t1ts[img]

                def t1i(m):
                    return t1t[:, m * IP + c0 : m * IP + c0 + n].rearrange(
                        "p (h w) -> p h w", h=rows
                    )[:, :, 1:57]

                nc.scalar.activation(
                    a2ts[img][:, OP + cc0 : OP + cc0 + nc_].rearrange(
                        "p (h w) -> p h w", h=rows
                    ),
                    t1i(1), AF.Sign, bias=cc(1, 6),
                )
                if not skip_ge:
                    nc.vector.tensor_scalar(
                        a2ts[img][:, cc0 : cc0 + nc_].rearrange(
                            "p (h w) -> p h w", h=rows
                        ),
                        t1i(0), cc(0, 1), None, Alu.is_ge,
                    )
                for mc in range(2):
                    nc.scalar.activation(
                        p1ts[img][:, mc * OP + cc0 : mc * OP + cc0 + nc_]
                        .rearrange("p (h w) -> p h w", h=rows),
                        t1i(mc), AF.Prelu, alpha=cc(mc, 3),
                    )

            def conv1_pair(img, tp):
                # tiles 2tp(,2tp+1) share one 2-bank PSUM tile per mc so the
                # DVE drains 928 columns per instruction
                if tp == 0:
                    t1ts[img] = t1pool.tile(
                        [128, 2 * IP], bf16, tag="t1", name=f"t1_{img}"
                    )
                    a2ts[img] = a2pool.tile(
                        [128, 2 * OP], fp8, tag="a2", name=f"a2_{img}"
                    )
                    p1ts[img] = p1pool.tile(
                        [128, 2 * OP], bf16, tag="p1", name=f"p1_{img}"
                    )
                t1t = t1ts[img]
                hv = hts[img][:].rearrange("p (two w) -> p two w", two=2)
                tiles = [2 * tp] if 2 * tp + 1 >= NT else [2 * tp, 2 * tp + 1]
                c0 = TN * 2 * tp
                n = TN * len(tiles)
                # last image: mc1 first, its Sign gates the tail conv2
                for mc in ((1, 0) if img == BPC - 1 else (0, 1)):
                    ps = ps1p.tile([128, 1024], f32, tag="ps1")
                    for j, t in enumerate(tiles):
                        for sh in range(9):
                            kh, kw = divmod(sh, 3)
                            off = HOFF + (8 * t + kh) * PH + (kw - 1)
                            nc.tensor.matmul(
                                ps[:, 512 * j : 512 * j + TN],
                                w3v[:, sh * 2 + mc],
                                hv[:, :, off : off + TN],
                                start=(sh == 0),
                                stop=(sh == 8),
                                perf_mode=DR,
                            )
                    # t1 = psum*(2*sh1) + xprep (x + bn1 + b12 - sh1*M1 folded)
                    if len(tiles) == 2:
                        psv = ps[:].rearrange(
                            "p (two w) -> p two w", two=2
                        )[:, :, :TN]
                        t1v = t1t[:, mc * IP + c0 : mc * IP + c0 + n].rearrange(
                            "p (two w) -> p two w", two=2
                        )
                        xv = xts[img][
                            :, mc * NPIX + PH + c0 : mc * NPIX + PH + c0 + n
                        ].rearrange("p (two w) -> p two w", two=2)
                        nc.vector.scalar_tensor_tensor(
                            t1v, psv, cc(mc, 0), xv, Alu.mult, Alu.add
                        )
                    else:
                        nc.vector.scalar_tensor_tensor(
                            t1t[:, mc * IP + c0 : mc * IP + c0 + n],
                            ps[:, :TN], cc(mc, 0),
                            xts[img][:, mc * NPIX + PH + c0 :
                                      mc * NPIX + PH + c0 + n],
                            Alu.mult, Alu.add,
                        )

            ots = [None] * BPC

            TC = 448  # compact tile width (8 rows x 56)

            def conv2_pair(img, tp):
                if tp == 0:
                    t2ts[img] = t2pool.tile(
                        [128, 2 * OP], bf16, tag="t2", name=f"t2_{img}"
                    )
                    ots[img] = opool.tile(
                        [128, 2 * OP], bf16, tag="o", name=f"o_{img}"
                    )
                t2t = t2ts[img]
                a2v = a2ts[img][:].rearrange("p (two w) -> p two w", two=2)
                tiles = [2 * tp] if 2 * tp + 1 >= NT else [2 * tp, 2 * tp + 1]
                c0 = TC * 2 * tp
                n = TC * len(tiles)
                for mc in range(2):
                    ps = ps2p.tile([128, 1024], f32, tag="ps2")
                    for j, t in enumerate(tiles):
                        nc.tensor.matmul(
                            ps[:, 512 * j : 512 * j + TC],
                            w1v[:, mc],
                            a2v[:, :, TC * t : TC * t + TC],
                            start=True,
                            stop=True,
                            perf_mode=DR,
                        )
                    # t2 = psum*sh2 + p1 on DVE; prelu2 chunk follows (ACT)
                    if len(tiles) == 2:
                        psv = ps[:].rearrange(
                            "p (two w) -> p two w", two=2
                        )[:, :, :TC]
                        t2v = t2t[:, mc * OP + c0 : mc * OP + c0 + n].rearrange(
                            "p (two w) -> p two w", two=2
                        )
                        pv = p1ts[img][
                            :, mc * OP + c0 : mc * OP + c0 + n
                        ].rearrange("p (two w) -> p two w", two=2)
                        nc.vector.scalar_tensor_tensor(
                            t2v, psv, cc(mc, 5), pv, Alu.mult, Alu.add
                        )
                    else:
                        nc.vector.scalar_tensor_tensor(
                            t2t[:, mc * OP + c0 : mc * OP + c0 + n],
                            ps[:, :TC], cc(mc, 5),
                            p1ts[img][:, mc * OP + c0 : mc * OP + c0 + n],
                            Alu.mult, Alu.add,
                        )

            def prelu2_tiles(img, t0, t1):
                cc0, nc_ = 448 * t0, 448 * (t1 - t0)
                for mc in range(2):
                    # out = prelu(t2 + K2', a2); b23 on host
                    nc.scalar.activation(
                        ots[img][:, mc * OP + cc0 : mc * OP + cc0 + nc_],
                        t2ts[img][:, mc * OP + cc0 : mc * OP + cc0 + nc_],
                        AF.Prelu, bias=cc(mc, 2), alpha=cc(mc, 4),
                    )

            def dma_out_tiles(img, t0, t1):
                b0, b1 = 448 * t0, 448 * t1
                for mc in range(2):
                    nc.sync.dma_start(
                        o_d.ap()[img, mc][:, b0:b1],
                        ots[img][:, mc * OP + b0 : mc * OP + b1],
                    )

            load(0)
            NP_ = (NT + 1) // 2  # 4 pair-slots
            LAST = BPC - 1
            for img in range(BPC):
                if img + 1 < BPC:
                    load(img + 1)
                last = img == LAST
                for tp in range(NP_):
                    conv1_pair(img, tp)
                    binarize_tiles(img, 2 * tp, min(2 * tp + 2, NT),
                                   skip_ge=last and tp < 3)
                    # one conv2 pair per slot: slot 0 hosts the previous
                    # image's last pair, slots 1-3 host this image's pairs
                    # 0-2 one slot behind their binarized data
                    if tp == 0:
                        if img >= 1:
                            conv2_pair(img - 1, 3)
                            prelu2_tiles(img - 1, 6, NT)
                            dma_out_tiles(img - 1, 6, NT)
                    else:
                        conv2_pair(img, tp - 1)
                        prelu2_tiles(img, 2 * (tp - 1), 2 * tp)
                        if last:
                            # stream the last image's output out as it is
                            # produced so the exit barrier never waits on
                            # a bulk DMA drain
                            dma_out_tiles(img, 2 * (tp - 1), 2 * tp)
                        elif tp == 3:
                            dma_out_tiles(img, 0, 6)
                    if last and tp < 3:
                        # last image: ge after the slot's stt2 drains; its
                        # consumer is a slot away, the PSUM banks are not.
                        # (pair 3 keeps ge inline: the tail conv2 gates on
                        # it immediately)
                        ge_tiles(img, 2 * tp, min(2 * tp + 2, NT))
            conv2_pair(LAST, 3)
            prelu2_tiles(LAST, 6, NT)
            dma_out_tiles(LAST, 6, NT)

    _CACHE["nc"] = nc
    return nc


def _host_fold(w3, w1, b11, b12, b13, b21, b22, b23,
               g1, be1, m1, v1, g2, be2, m2, v2, a1, a2):
    f = np.float32
    s3 = np.mean(np.abs(w3), axis=(1, 2, 3)).astype(f)
    s1 = np.mean(np.abs(w1), axis=(1, 2, 3)).astype(f)
    inv1 = (g1 / np.sqrt(v1 + EPS)).astype(f)
    inv2 = (g2 / np.sqrt(v2 + EPS)).astype(f)
    sh1 = s3 * inv1
    ch1 = be1 - m1 * inv1
    sh2 = s1 * inv2
    ch2 = be2 - m2 * inv2
    K1 = (ch1 + b12).astype(f)

    sgn3 = np.sign(w3).astype(f)                     # [O, I, 3, 3]
    sgn1 = np.sign(w1).astype(f)
    # M1[c, i, j] = sum over in-bounds taps of rowsum3[c, kh, kw]
    rowsum3 = sgn3.sum(axis=1)                       # [C, 3, 3]
    M1 = np.zeros((C, H, W), f)
    for kh in range(3):
        for kw in range(3):
            ind = np.zeros((H, W), f)
            r0, r1 = max(0, 1 - kh), min(H - 1, H - kh) + 1
            c0, c1 = max(0, 1 - kw), min(W - 1, W - kw) + 1
            ind[r0:r1, c0:c1] = 1.0
            M1 += rowsum3[:, kh, kw][:, None, None] * ind[None]
    xadj = K1[:, None, None] - sh1[:, None, None] * M1   # [C, H, W]

    # A2 threshold: p1 >= -(b13+b21)  <=>  t1 >= thr (prelu inverse, a1>=0)
    u = (-(b13 + b21)).astype(f)
    safe_a1 = np.where(a1 > 0, a1, 1.0).astype(f)
    thr = np.where(u > 0, u, np.where(a1 > 0, u / safe_a1, f(-3e38)))
    # {0,1}-encoded kc0 half: conv(2H-1) correction = rowsum over kc0 inputs
    r1h = sgn1[:, :128].sum(axis=(1, 2, 3)).astype(f)
    K2p = (ch2 + b13 + b22 - sh2 * r1h).astype(f)

    fp8 = ml_dtypes.float8_e4m3
    # DoubleRow lhsT layout: [k, ((sh*2+mc)*2+i)*128+m] with i the K-half
    W3 = sgn3.astype(fp8)                                       # [O, I, 3, 3]
    W3 = W3.reshape(2, 128, 2, 128, 3, 3)                       # [mc,m,i,k,kh,kw]
    W3 = W3.transpose(3, 4, 5, 0, 2, 1)                         # [k,kh,kw,mc,i,m]
    W3f = np.ascontiguousarray(W3.reshape(128, 9 * 2 * 2 * 128))
    W1 = sgn1.reshape(2, 128, 2, 128).copy()                    # [mc, m, i, k]
    W1[:, :, 0, :] *= 2.0              # kc0 half is {0,1}-encoded: +-2
    W1 = W1.astype(fp8).transpose(3, 0, 2, 1)                   # [k, mc, i, m]
    W1f = np.ascontiguousarray(W1.reshape(128, 2 * 2 * 128))

    consts = np.zeros((2, 128, 8), f)
    for kc in range(2):
        sl = slice(kc * 128, (kc + 1) * 128)
        consts[kc, :, 0] = 2.0 * sh1[sl]
        consts[kc, :, 1] = thr[sl]
        consts[kc, :, 2] = K2p[sl]
        consts[kc, :, 3] = a1[sl]
        consts[kc, :, 4] = a2[sl]
        consts[kc, :, 5] = sh2[sl]
        consts[kc, :, 6] = -thr[sl]
    return W3f, W1f, consts, xadj


def _run(in_maps, trace=False, tmpdir=None, trace_kwargs={}):
    from concourse import bass_utils

    nc = build_nc()
    if not _CACHE.get("split"):
        # walrus workaround applied only for the HW path (CoreSim rejects
        # post-scheduling instruction edits)
        nc.m = _split_drain_waits(nc.m)
        _CACHE["split"] = True
    return bass_utils.run_bass_kernel_spmd(
        nc,
        in_maps,
        core_ids=list(range(NCORES)),
        trace=trace,
        tmpdir=tmpdir,
        trace_kwargs=trace_kwargs,
    )


def make_in_maps(x, w3, w1, **params):
    x = np.asarray(x, np.float32)
    params = {k: np.asarray(v, np.float32) for k, v in params.items()}
    W3f, W1f, consts, xadj = _host_fold(np.asarray(w3, np.float32),
                                        np.asarray(w1, np.float32), **params)
    _CACHE["b23"] = params["b23"]
    bf16 = ml_dtypes.bfloat16
    fp8 = ml_dtypes.float8_e4m3
    N = x.shape[0]

    xp = np.zeros((N, C, 58, PH), bf16)
    xp[:, :, 1:57, 1:57] = (x + xadj[None]).astype(bf16)
    x_prep = xp.reshape(N, 2, 128, NPIX)

    hp = np.zeros((N, C, 58, PH), fp8)
    hp[:, :, 1:57, 1:57] = (
        x + params["b11"][None, :, None, None] >= 0
    ).astype(fp8)
    harr = np.zeros((N, 128, 2, H1BLK), fp8)
    harr[:, :, :, HOFF : HOFF + NPIX] = hp.reshape(
        N, 2, 128, NPIX
    ).transpose(0, 2, 1, 3)
    harr = harr.reshape(N, 128, 2 * H1BLK)

    return [
        {
            "xprep": np.ascontiguousarray(x_prep[c * BPC : (c + 1) * BPC]),
            "h1": np.ascontiguousarray(harr[c * BPC : (c + 1) * BPC]),
            "w3f": W3f, "w1f": W1f, "consts": consts,
        }
        for c in range(NCORES)
    ]


def finish_out(arr):
    """Per-core raw out [BPC,2,128,OP] bf16 -> [BPC,C,H,W] fp32 (+b23)."""
    out = np.asarray(arr).reshape(BPC, C, H, W).astype(np.float32)
    return out + _CACHE["b23"][None, :, None, None]


def assemble_out(results):
    outs = [finish_out(results[c]["out"]) for c in range(NCORES)]
    return np.ascontiguousarray(np.concatenate(outs, axis=0))


def _fallback_numpy(x, w3, w1, b11, b12, b13, b21, b22, b23,
                    g1, be1, m1, v1, g2, be2, m2, v2, a1, a2):
    # Straightforward reference math in numpy; only used if an assumption of
    # the device kernel (prelu slope a1 >= 0) is violated.
    def cb(p):
        return p[None, :, None, None]

    def conv_np(a, w, pad):
        N, Ci, Hh, Ww = a.shape
        O, I, kh, kw = w.shape
        ap = np.pad(a, ((0, 0), (0, 0), (pad, pad), (pad, pad)))
        out = np.zeros((N, O, Hh, Ww), np.float32)
        wm = w.reshape(O, -1)
        for n in range(N):
            cols = np.empty((I * kh * kw, Hh * Ww), np.float32)
            idx = 0
            for i in range(I):
                for dh in range(kh):
                    for dw in range(kw):
                        cols[idx] = ap[n, i, dh : dh + Hh, dw : dw + Ww].ravel()
                        idx += 1
            out[n] = (wm @ cols).reshape(O, Hh, Ww)
        return out

    def bn(t, g, b, mm, v):
        inv = g / np.sqrt(v + EPS)
        return t * cb(inv) + cb(b - mm * inv)

    def prelu(t, a):
        return np.where(t > 0, t, cb(a) * t)

    s3 = np.mean(np.abs(w3), axis=(1, 2, 3), keepdims=True)
    s1 = np.mean(np.abs(w1), axis=(1, 2, 3), keepdims=True)
    o1 = conv_np(np.sign(x + cb(b11)), np.sign(w3) * s3, 1)
    o1 = x + bn(o1, g1, be1, m1, v1)
    o1 = prelu(o1 + cb(b12), a1) + cb(b13)
    o2 = conv_np(np.sign(o1 + cb(b21)), np.sign(w1) * s1, 0)
    o2 = bn(o2, g2, be2, m2, v2) + o1
    o2 = prelu(o2 + cb(b22), a2) + cb(b23)
    return o2.astype(np.float32)


def kernel(**inputs):
    inputs = {k: np.asarray(v) for k, v in inputs.items()}
    if (np.asarray(inputs["a1"], np.float32) < 0).any():
        return _fallback_numpy(**{k: np.asarray(v, np.float32)
                                  for k, v in inputs.items()})
    in_maps = make_in_maps(**inputs)
    res = _run(in_maps, trace=False)
    return assemble_out(res.results)
